# revision 1
# baseline (speedup 1.0000x reference)
"""MoE gate (LLaDA2) routing kernel for 8 Trainium2 NeuronCores.

Strategy: token-parallel over 8 cores (2048 tokens/core). Router GEMM as
fp16 3-term split (xhi@whi + xhi@wlo + xlo@whi, fp32 PSUM accumulate) which
matches fp32 reference selection at its accumulation-noise floor. Grouped
top-k routing done on-chip with DVE max8/max_index/match_replace.
"""
import sys
for p in ("/opt/trn_rl_repo", "/root/.axon_site/_ro/trn_rl_repo"):
    if p not in sys.path:
        sys.path.append(p)

import numpy as np

T, H, E = 16384, 4096, 256
NCORES = 8
TPC = T // NCORES          # tokens per core: 2048
NTILES = TPC // 128        # 16 row tiles
KCH = H // 128             # 32 contraction chunks
G = 8                      # expert groups
GS = E // G                # 32 experts/group
K = 8                      # top-k
BIG = 2.0 ** 100
NEG = -1.0e4

_cache = {}


def _build():
    import concourse.bacc as bacc
    import concourse.bass as bass
    import concourse.mybir as mybir
    from concourse import tile

    dt = mybir.dt
    Alu = mybir.AluOpType
    Act = mybir.ActivationFunctionType
    Ax = mybir.AxisListType

    nc = bacc.Bacc("TRN2", target_bir_lowering=False, debug=False,
                   num_devices=NCORES)

    xhi_d = nc.dram_tensor("xhi", [NTILES, 128, KCH, 128], dt.float16, kind="ExternalInput")
    xlo_d = nc.dram_tensor("xlo", [NTILES, 128, KCH, 128], dt.float16, kind="ExternalInput")
    whi_d = nc.dram_tensor("whi", [128, KCH, E], dt.float16, kind="ExternalInput")
    wlo_d = nc.dram_tensor("wlo", [128, KCH, E], dt.float16, kind="ExternalInput")
    btab_d = nc.dram_tensor("btab", [128, E], dt.float32, kind="ExternalInput")
    w_out = nc.dram_tensor("w_out", [TPC, K], dt.float32, kind="ExternalOutput")
    i_out = nc.dram_tensor("i_out", [TPC, K], dt.uint32, kind="ExternalOutput")

    def bc_mid(ap8, n=8):
        # [128, m] -> [128, n(bcast), m]
        return bass.AP(ap8.tensor, ap8.offset, [list(ap8.ap[0]), [0, n], list(ap8.ap[1])])

    with tile.TileContext(nc) as tc:
        with (
            tc.tile_pool(name="wpool", bufs=1) as wpool,
            tc.tile_pool(name="xpool", bufs=3) as xpool,
            tc.tile_pool(name="ppool", bufs=4, space="PSUM") as ppool,
            tc.tile_pool(name="spool", bufs=2) as spool,
            tc.tile_pool(name="tpool", bufs=2) as tpool,
            tc.tile_pool(name="opool", bufs=1) as opool,
        ):
            whi = wpool.tile([128, KCH * E], dt.float16, tag="whi")
            wlo = wpool.tile([128, KCH * E], dt.float16, tag="wlo")
            btab = wpool.tile([128, E], dt.float32, tag="btab")
            nc.sync.dma_start(whi[:], whi_d[:].rearrange("p k e -> p (k e)"))
            nc.sync.dma_start(wlo[:], wlo_d[:].rearrange("p k e -> p (k e)"))
            nc.sync.dma_start(btab[:], btab_d[:])

            out_w = opool.tile([128, NTILES * K], dt.float32, tag="ow")
            out_i = opool.tile([128, NTILES * K], dt.uint32, tag="oi")

            for i in range(NTILES):
                xhi = xpool.tile([128, KCH * 128], dt.float16, tag="xhi")
                xlo = xpool.tile([128, KCH * 128], dt.float16, tag="xlo")
                nc.sync.dma_start(xhi[:], xhi_d[i].rearrange("p k t -> p (k t)"))
                nc.sync.dma_start(xlo[:], xlo_d[i].rearrange("p k t -> p (k t)"))

                psum = ppool.tile([128, E], dt.float32, tag="ps")
                n_mm = KCH * 3
                mm = 0
                for k in range(KCH):
                    xh = xhi[:, k * 128:(k + 1) * 128]
                    xl = xlo[:, k * 128:(k + 1) * 128]
                    wh = whi[:, k * E:(k + 1) * E]
                    wl = wlo[:, k * E:(k + 1) * E]
                    for lhsT, rhs in ((xh, wh), (xh, wl), (xl, wh)):
                        nc.tensor.matmul(psum[:], lhsT=lhsT, rhs=rhs,
                                         start=(mm == 0), stop=(mm == n_mm - 1))
                        mm += 1

                # --- routing epilogue ---
                scores = spool.tile([128, E], dt.float32, tag="scores")
                nc.scalar.activation(scores[:], psum[:], Act.Sigmoid)

                sr = spool.tile([128, E], dt.float32, tag="sr")
                nc.vector.tensor_tensor(sr[:], scores[:], btab[:], Alu.add)
                sr3 = sr[:].rearrange("p (g e) -> p g e", g=G)

                top1 = tpool.tile([128, G], dt.float32, tag="top1")
                nc.vector.tensor_reduce(top1[:], sr3, axis=Ax.X, op=Alu.max)
                mr2 = spool.tile([128, E], dt.float32, tag="mr2")
                nc.vector.match_replace(mr2[:], in_to_replace=top1[:], in_values=sr[:], imm_value=NEG)
                top2 = tpool.tile([128, G], dt.float32, tag="top2")
                nc.vector.tensor_reduce(top2[:], mr2[:].rearrange("p (g e) -> p g e", g=G), axis=Ax.X, op=Alu.max)
                gs_t = tpool.tile([128, G], dt.float32, tag="gs")
                nc.vector.tensor_tensor(gs_t[:], top1[:], top2[:], Alu.add)
                g8 = tpool.tile([128, 8], dt.float32, tag="g8")
                nc.vector.max(out=g8[:], in_=gs_t[:])
                inv = tpool.tile([128, G], dt.float32, tag="inv")
                nc.vector.tensor_scalar(inv[:], gs_t[:], g8[:, 3:4], -NEG, op0=Alu.is_lt, op1=Alu.mult)
                # mask: sr -= inv (0 for kept groups, 1e4 for dropped)
                nc.vector.tensor_tensor(sr3, sr3, inv[:].to_broadcast([128, G, GS]), Alu.subtract)

                vals8 = tpool.tile([128, K], dt.float32, tag="vals8")
                nc.vector.max(out=vals8[:], in_=sr[:])
                idx8 = tpool.tile([128, K], dt.uint32, tag="idx8")
                nc.vector.max_index(out=idx8[:], in_max=vals8[:], in_values=sr[:])

                mr = spool.tile([128, E], dt.float32, tag="mr")
                nc.vector.match_replace(mr[:], in_to_replace=vals8[:], in_values=sr[:], imm_value=BIG)
                diff = spool.tile([128, E], dt.float32, tag="diff")
                nc.vector.tensor_tensor(diff[:], mr[:], sr[:], Alu.subtract)
                sel = spool.tile([128, E], dt.float32, tag="sel")
                nc.scalar.mul(sel[:], diff[:], 2.0 ** -100)
                sel_s = spool.tile([128, E], dt.float32, tag="sel_s")
                nc.vector.tensor_tensor(sel_s[:], scores[:], sel[:], Alu.mult)

                svals8 = tpool.tile([128, K], dt.float32, tag="svals8")
                nc.vector.max(out=svals8[:], in_=sel_s[:])
                sidx8 = tpool.tile([128, K], dt.uint32, tag="sidx8")
                nc.vector.max_index(out=sidx8[:], in_max=svals8[:], in_values=sel_s[:])

                idx8f = tpool.tile([128, K], dt.float32, tag="idx8f")
                nc.vector.tensor_copy(idx8f[:], idx8[:])
                sidx8f = tpool.tile([128, K], dt.float32, tag="sidx8f")
                nc.vector.tensor_copy(sidx8f[:], sidx8[:])

                eq = tpool.tile([128, K * K], dt.float32, tag="eq")
                eq3 = eq[:].rearrange("p (k j) -> p k j", k=K)
                nc.vector.tensor_tensor(eq3, idx8f[:].to_broadcast([128, K, K]), bc_mid(sidx8f[:]), Alu.is_equal)
                prod = tpool.tile([128, K * K], dt.float32, tag="prod")
                prod3 = prod[:].rearrange("p (k j) -> p k j", k=K)
                nc.vector.tensor_tensor(prod3, eq3, bc_mid(svals8[:]), Alu.mult)
                w8 = tpool.tile([128, K], dt.float32, tag="w8")
                nc.vector.tensor_reduce(w8[:], prod3, axis=Ax.X, op=Alu.add)

                sum8 = tpool.tile([128, 1], dt.float32, tag="sum8")
                nc.vector.tensor_reduce(sum8[:], w8[:], axis=Ax.X, op=Alu.add)
                rec = tpool.tile([128, 1], dt.float32, tag="rec")
                nc.vector.reciprocal(rec[:], sum8[:])

                nc.vector.tensor_scalar(out_w[:, i * K:(i + 1) * K], w8[:], rec[:, 0:1], 2.5,
                                        op0=Alu.mult, op1=Alu.mult)
                nc.vector.tensor_copy(out_i[:, i * K:(i + 1) * K], idx8[:])

            nc.sync.dma_start(w_out[:].rearrange("(i p) k -> p i k", p=128),
                              out_w[:].rearrange("p (i k) -> p i k", i=NTILES))
            nc.sync.dma_start(i_out[:].rearrange("(i p) k -> p i k", p=128),
                              out_i[:].rearrange("p (i k) -> p i k", i=NTILES))

    nc.compile()
    return nc


def _prep(hidden_states, weight, expert_bias):
    x = np.ascontiguousarray(hidden_states, dtype=np.float32)
    w = np.ascontiguousarray(weight, dtype=np.float32)
    whi = w.astype(np.float16)
    wlo = (w - whi.astype(np.float32)).astype(np.float16)
    # [256, 4096] -> [128p, 32k, 256e]
    whi_l = np.ascontiguousarray(whi.reshape(E, KCH, 128).transpose(2, 1, 0))
    wlo_l = np.ascontiguousarray(wlo.reshape(E, KCH, 128).transpose(2, 1, 0))
    btab = np.ascontiguousarray(np.broadcast_to(expert_bias.astype(np.float32), (128, E)))

    in_maps = []
    for c in range(NCORES):
        xs = x[c * TPC:(c + 1) * TPC]
        xhi = xs.astype(np.float16)
        xlo = (xs - xhi.astype(np.float32)).astype(np.float16)
        # [2048, 4096] -> [16i, 128p(h), 32k, 128t]
        xhi_l = np.ascontiguousarray(xhi.reshape(NTILES, 128, KCH, 128).transpose(0, 3, 2, 1))
        xlo_l = np.ascontiguousarray(xlo.reshape(NTILES, 128, KCH, 128).transpose(0, 3, 2, 1))
        in_maps.append({"xhi": xhi_l, "xlo": xlo_l, "whi": whi_l, "wlo": wlo_l, "btab": btab})
    return in_maps


def kernel(hidden_states, weight, expert_bias, _trace=False):
    from concourse.bass_utils import run_bass_kernel_spmd

    if "nc" not in _cache:
        _cache["nc"] = _build()
    nc = _cache["nc"]
    in_maps = _prep(hidden_states, weight, expert_bias)
    res = run_bass_kernel_spmd(nc, in_maps, core_ids=list(range(NCORES)), trace=_trace)
    _cache["last_results"] = res
    w = np.concatenate([res.results[c]["w_out"] for c in range(NCORES)], axis=0)
    idx = np.concatenate([res.results[c]["i_out"] for c in range(NCORES)], axis=0)
    return w.astype(np.float32), idx.astype(np.int32)



# revision 2
# speedup vs baseline: 1.3774x; 1.3774x over previous
"""MoE gate (LLaDA2) routing kernel for 8 Trainium2 NeuronCores.

Strategy: token-parallel over 8 cores (2048 tokens/core). Router GEMM as a
single-pass float32r matmul (1 cycle/row on TRN2 when the moving free dim is
>= 256, here exactly E=256) — full fp32 operand precision, 3x less PE time
than an fp16 hi/lo 3-term split. Grouped top-k routing on-chip with DVE
max8/max_index/match_replace, fused where possible (max_with_indices,
scalar_tensor_tensor).
"""
import sys
for p in ("/opt/trn_rl_repo", "/root/.axon_site/_ro/trn_rl_repo"):
    if p not in sys.path:
        sys.path.append(p)

import numpy as np

T, H, E = 16384, 4096, 256
NCORES = 8
TPC = T // NCORES          # tokens per core: 2048
NTILES = TPC // 128        # 16 row tiles
KCH = H // 128             # 32 contraction chunks
G = 8                      # expert groups
GS = E // G                # 32 experts/group
K = 8                      # top-k
BIG = 2.0 ** 100
NEG = -1.0e4
W_SPLITS = 8               # parallel DMA slices for the weight load

_cache = {}


def _build():
    import concourse.bacc as bacc
    import concourse.bass as bass
    import concourse.mybir as mybir
    from concourse import tile

    dt = mybir.dt
    Alu = mybir.AluOpType
    Act = mybir.ActivationFunctionType
    Ax = mybir.AxisListType

    nc = bacc.Bacc("TRN2", target_bir_lowering=False, debug=False,
                   num_devices=NCORES)

    x_d = nc.dram_tensor("x", [NTILES, 128, KCH, 128], dt.float32r, kind="ExternalInput")
    w_d = nc.dram_tensor("w", [128, KCH, E], dt.float32r, kind="ExternalInput")
    btab_d = nc.dram_tensor("btab", [128, E], dt.float32, kind="ExternalInput")
    w_out = nc.dram_tensor("w_out", [TPC, K], dt.float32, kind="ExternalOutput")
    i_out = nc.dram_tensor("i_out", [TPC, K], dt.uint32, kind="ExternalOutput")

    def bc_mid(ap8, n=8):
        # [128, m] -> [128, n(bcast), m]
        return bass.AP(ap8.tensor, ap8.offset, [list(ap8.ap[0]), [0, n], list(ap8.ap[1])])

    with tile.TileContext(nc) as tc:
        with (
            tc.tile_pool(name="wpool", bufs=1) as wpool,
            tc.tile_pool(name="xpool", bufs=3) as xpool,
            tc.tile_pool(name="ppool", bufs=4, space="PSUM") as ppool,
            tc.tile_pool(name="spool", bufs=2) as spool,
            tc.tile_pool(name="tpool", bufs=2) as tpool,
            tc.tile_pool(name="opool", bufs=1) as opool,
        ):
            w = wpool.tile([128, KCH * E], dt.float32r, tag="w")
            btab = wpool.tile([128, E], dt.float32, tag="btab")
            # split the 32KB/partition weight load across DMA queues
            wsz = KCH * E // W_SPLITS
            w_flat = w_d[:].rearrange("p k e -> p (k e)")
            for s in range(W_SPLITS):
                nc.sync.dma_start(w[:, s * wsz:(s + 1) * wsz],
                                  w_flat[:, s * wsz:(s + 1) * wsz])
            nc.sync.dma_start(btab[:], btab_d[:])

            out_w = opool.tile([128, NTILES * K], dt.float32, tag="ow")
            out_i = opool.tile([128, NTILES * K], dt.uint32, tag="oi")

            for i in range(NTILES):
                x = xpool.tile([128, KCH * 128], dt.float32r, tag="x")
                nc.sync.dma_start(x[:], x_d[i].rearrange("p k t -> p (k t)"))

                psum = ppool.tile([128, E], dt.float32, tag="ps")
                for k in range(KCH):
                    nc.tensor.matmul(psum[:],
                                     lhsT=x[:, k * 128:(k + 1) * 128],
                                     rhs=w[:, k * E:(k + 1) * E],
                                     start=(k == 0), stop=(k == KCH - 1))

                # --- routing epilogue ---
                scores = spool.tile([128, E], dt.float32, tag="scores")
                nc.scalar.activation(scores[:], psum[:], Act.Sigmoid)

                sr = spool.tile([128, E], dt.float32, tag="sr")
                nc.vector.tensor_tensor(sr[:], scores[:], btab[:], Alu.add)
                sr3 = sr[:].rearrange("p (g e) -> p g e", g=G)

                top1 = tpool.tile([128, G], dt.float32, tag="top1")
                nc.vector.tensor_reduce(top1[:], sr3, axis=Ax.X, op=Alu.max)
                mr2 = spool.tile([128, E], dt.float32, tag="mr2")
                nc.vector.match_replace(mr2[:], in_to_replace=top1[:], in_values=sr[:], imm_value=NEG)
                top2 = tpool.tile([128, G], dt.float32, tag="top2")
                nc.vector.tensor_reduce(top2[:], mr2[:].rearrange("p (g e) -> p g e", g=G), axis=Ax.X, op=Alu.max)
                gs_t = tpool.tile([128, G], dt.float32, tag="gs")
                nc.vector.tensor_tensor(gs_t[:], top1[:], top2[:], Alu.add)
                g8 = tpool.tile([128, 8], dt.float32, tag="g8")
                nc.vector.max(out=g8[:], in_=gs_t[:])
                inv = tpool.tile([128, G], dt.float32, tag="inv")
                nc.vector.tensor_scalar(inv[:], gs_t[:], g8[:, 3:4], -NEG, op0=Alu.is_lt, op1=Alu.mult)
                # mask: sr -= inv (0 for kept groups, 1e4 for dropped)
                nc.vector.tensor_tensor(sr3, sr3, inv[:].to_broadcast([128, G, GS]), Alu.subtract)

                vals8 = tpool.tile([128, K], dt.float32, tag="vals8")
                idx8 = tpool.tile([128, K], dt.uint32, tag="idx8")
                nc.vector.max_with_indices(out_max=vals8[:], out_indices=idx8[:], in_=sr[:])

                # selected positions -> exactly BIG; (BIG * 2^-100) * score = score
                mr = spool.tile([128, E], dt.float32, tag="mr")
                nc.vector.match_replace(mr[:], in_to_replace=vals8[:], in_values=sr[:], imm_value=BIG)
                sel_s = spool.tile([128, E], dt.float32, tag="sel_s")
                nc.vector.scalar_tensor_tensor(sel_s[:], in0=mr[:], scalar=2.0 ** -100,
                                               in1=scores[:], op0=Alu.mult, op1=Alu.mult)

                svals8 = tpool.tile([128, K], dt.float32, tag="svals8")
                sidx8 = tpool.tile([128, K], dt.uint32, tag="sidx8")
                nc.vector.max_with_indices(out_max=svals8[:], out_indices=sidx8[:], in_=sel_s[:])

                idx8f = tpool.tile([128, K], dt.float32, tag="idx8f")
                nc.vector.tensor_copy(idx8f[:], idx8[:])
                sidx8f = tpool.tile([128, K], dt.float32, tag="sidx8f")
                nc.vector.tensor_copy(sidx8f[:], sidx8[:])

                # reorder svals8 (score order) into idx8 (routing order): K x K match
                eq = tpool.tile([128, K * K], dt.float32, tag="eq")
                eq3 = eq[:].rearrange("p (k j) -> p k j", k=K)
                nc.vector.tensor_tensor(eq3, idx8f[:].to_broadcast([128, K, K]), bc_mid(sidx8f[:]), Alu.is_equal)
                prod = tpool.tile([128, K * K], dt.float32, tag="prod")
                prod3 = prod[:].rearrange("p (k j) -> p k j", k=K)
                nc.vector.tensor_tensor(prod3, eq3, bc_mid(svals8[:]), Alu.mult)
                w8 = tpool.tile([128, K], dt.float32, tag="w8")
                nc.vector.tensor_reduce(w8[:], prod3, axis=Ax.X, op=Alu.add)

                sum8 = tpool.tile([128, 1], dt.float32, tag="sum8")
                nc.vector.tensor_reduce(sum8[:], w8[:], axis=Ax.X, op=Alu.add)
                rec = tpool.tile([128, 1], dt.float32, tag="rec")
                nc.vector.reciprocal(rec[:], sum8[:])

                nc.vector.tensor_scalar(out_w[:, i * K:(i + 1) * K], w8[:], rec[:, 0:1], 2.5,
                                        op0=Alu.mult, op1=Alu.mult)
                nc.vector.tensor_copy(out_i[:, i * K:(i + 1) * K], idx8[:])

            nc.sync.dma_start(w_out[:].rearrange("(i p) k -> p i k", p=128),
                              out_w[:].rearrange("p (i k) -> p i k", i=NTILES))
            nc.sync.dma_start(i_out[:].rearrange("(i p) k -> p i k", p=128),
                              out_i[:].rearrange("p (i k) -> p i k", i=NTILES))

    nc.compile()
    return nc


def _prep(hidden_states, weight, expert_bias):
    x = np.ascontiguousarray(hidden_states, dtype=np.float32)
    w = np.ascontiguousarray(weight, dtype=np.float32)
    # [256, 4096] -> [128p, 32k, 256e]
    w_l = np.ascontiguousarray(w.reshape(E, KCH, 128).transpose(2, 1, 0))
    btab = np.ascontiguousarray(np.broadcast_to(expert_bias.astype(np.float32), (128, E)))

    in_maps = []
    for c in range(NCORES):
        xs = x[c * TPC:(c + 1) * TPC]
        # [2048, 4096] -> [16i, 128p(h), 32k, 128t]
        x_l = np.ascontiguousarray(xs.reshape(NTILES, 128, KCH, 128).transpose(0, 3, 2, 1))
        in_maps.append({"x": x_l, "w": w_l, "btab": btab})
    return in_maps


def kernel(hidden_states, weight, expert_bias, _trace=False):
    from concourse.bass_utils import run_bass_kernel_spmd

    if "nc" not in _cache:
        _cache["nc"] = _build()
    nc = _cache["nc"]
    in_maps = _prep(hidden_states, weight, expert_bias)
    res = run_bass_kernel_spmd(nc, in_maps, core_ids=list(range(NCORES)), trace=_trace)
    _cache["last_results"] = res
    w = np.concatenate([res.results[c]["w_out"] for c in range(NCORES)], axis=0)
    idx = np.concatenate([res.results[c]["i_out"] for c in range(NCORES)], axis=0)
    return w.astype(np.float32), idx.astype(np.int32)
